# revision 11
# baseline (speedup 1.0000x reference)
import sys

import numpy as np

try:
    from concourse import bacc, bass, tile, mybir
    from concourse.bass_utils import run_bass_kernel_spmd
except ImportError:  # pragma: no cover
    sys.path.insert(0, "/opt/trn_rl_repo")
    from concourse import bacc, bass, tile, mybir
    from concourse.bass_utils import run_bass_kernel_spmd

B = 256        # batch
N = 16384      # neurons
DEG = 32       # fan-in per neuron
NCORES = 8
NPC = N // NCORES      # neurons per core = 2048
CH = 64                # neurons per chunk
NCHUNK = NPC // CH     # 32
IPC = CH * DEG         # gather indices per chunk = 2048
SW = IPC // 16         # wrapped idx cols per chunk = 128
G = 2                  # batch groups of 128

F32 = mybir.dt.float32
I16 = mybir.dt.int16

_CACHE = {}
TRACE = False
LAST_EXEC_TIME_NS = None
LAST_RESULTS = None


def _build():
    nc = bacc.Bacc("TRN2", target_bir_lowering=False)
    x_il = nc.dram_tensor([128, G * N], F32, kind="ExternalInput")
    idx_w = nc.dram_tensor([128, NCHUNK * SW], I16, kind="ExternalInput")
    w_wr = nc.dram_tensor([NCHUNK, IPC], F32, kind="ExternalInput")
    out_d = nc.dram_tensor([128, NCHUNK * G * CH], F32, kind="ExternalOutput")

    with tile.TileContext(nc) as tc:
        with (
            tc.tile_pool(name="xp", bufs=1) as xp,
            tc.tile_pool(name="cst", bufs=1) as cst,
            tc.tile_pool(name="gath", bufs=2) as gath,
            tc.tile_pool(name="red", bufs=2) as red,
            tc.tile_pool(name="outp", bufs=2) as outp,
            tc.tile_pool(name="wrow", bufs=3) as wrow,
            tc.tile_pool(name="psum", bufs=2, space=bass.MemorySpace.PSUM) as psump,
        ):
            xs = xp.tile([128, G * N], F32)
            idxs = cst.tile([128, NCHUNK * SW], I16)
            one_t = cst.tile([1, 128], F32)

            nc.vector.memset(one_t[:], 1.0)
            qs = (nc.sync, nc.scalar, nc.gpsimd)
            nseg = 3
            seg = G * N // nseg  # not exact; last segment padded below
            bounds = [0, seg, 2 * seg, G * N]
            for q in range(nseg):
                qs[q].dma_start(xs[:, bounds[q]:bounds[q + 1]],
                                x_il[:, bounds[q]:bounds[q + 1]])
            nc.gpsimd.dma_start(idxs[:], idx_w[:])

            xs3 = xs[:].rearrange("p (n g) -> p n g", g=G)
            for c in range(NCHUNK):
                # w chunk row -> partition 0, then PE-broadcast to 128 partitions
                wt = wrow.tile([1, IPC], F32)
                nc.gpsimd.dma_start(wt[:], w_wr[c:c + 1, :])
                pw = psump.tile([128, IPC], F32)
                for k in range(IPC // 512):
                    nc.tensor.matmul(
                        pw[:, k * 512:(k + 1) * 512],
                        one_t[:],
                        wt[0:1, k * 512:(k + 1) * 512],
                        start=True, stop=True,
                    )
                # gather: gt[p, j, g] = x[g*128+p, L[c*IPC+j]]
                gt = gath.tile([128, IPC * G], F32)
                nc.gpsimd.ap_gather(
                    gt[:], xs3, idxs[:, c * SW:(c + 1) * SW],
                    channels=128, num_elems=N, d=G, num_idxs=IPC,
                )
                # gt *= w (in place), w broadcast over g
                gt3 = gt[:].rearrange("p (j g) -> p j g", g=G)
                pwb = pw[:].unsqueeze(2).broadcast_to([128, IPC, G])
                nc.vector.tensor_tensor(gt3, gt3, pwb, mybir.AluOpType.mult)
                # reduce over fan-in d=32 (stride-2 inner): [128, i=64, g=2]
                ro = red.tile([128, CH * G], F32)
                nc.vector.tensor_reduce(
                    ro[:].rearrange("p (i g) -> p i g", g=G),
                    gt[:].rearrange("p (i d g) -> p i g d", d=DEG, g=G),
                    axis=mybir.AxisListType.X,
                    op=mybir.AluOpType.add,
                )
                # relu + deinterleave to g-major [p, g*64+i]
                oo = outp.tile([128, CH * G], F32)
                nc.scalar.activation(
                    oo[:].rearrange("p (g i) -> p i g", i=CH),
                    ro[:].rearrange("p (i g) -> p i g", g=G),
                    func=mybir.ActivationFunctionType.Relu,
                )
                qs[c % 3].dma_start(out_d[:, c * G * CH:(c + 1) * G * CH], oo[:])

    nc.finalize()
    names = dict(x_il=x_il.name, idx_w=idx_w.name, w_wr=w_wr.name, out=out_d.name)
    return nc, names


def _prep(x, w, idx, names):
    x = np.ascontiguousarray(np.asarray(x, dtype=np.float32))
    w = np.asarray(w, dtype=np.float32)
    idx = np.asarray(idx)
    # x_il[p, n*2+g] = x[g*128+p, n]
    x_il = np.ascontiguousarray(
        x.reshape(G, 128, N).transpose(1, 2, 0).reshape(128, G * N))
    in_maps = []
    for m in range(NCORES):
        sl = slice(m * NPC, (m + 1) * NPC)
        li = np.ascontiguousarray(idx[sl]).astype(np.int16).reshape(-1)
        iw = np.ascontiguousarray(np.tile(li.reshape(NCHUNK * SW, 16).T, (8, 1)))
        w_wr = np.ascontiguousarray(w[sl].reshape(NCHUNK, IPC))
        in_maps.append({names["x_il"]: x_il, names["idx_w"]: iw,
                        names["w_wr"]: w_wr})
    return in_maps


def kernel(x, w, idx):
    global LAST_EXEC_TIME_NS, LAST_RESULTS
    if "prog" not in _CACHE:
        _CACHE["prog"] = _build()
    nc, names = _CACHE["prog"]
    in_maps = _prep(x, w, idx, names)
    res = run_bass_kernel_spmd(nc, in_maps, core_ids=list(range(NCORES)),
                               trace=TRACE)
    LAST_EXEC_TIME_NS = res.exec_time_ns
    LAST_RESULTS = res
    outs = []
    for m in range(NCORES):
        o = np.asarray(res.results[m][names["out"]])
        o = o.reshape(128, NCHUNK, G, CH).transpose(2, 0, 1, 3).reshape(B, NPC)
        outs.append(o)
    return np.ascontiguousarray(np.concatenate(outs, axis=1)).astype(np.float32)


# revision 15
# speedup vs baseline: 1.0030x; 1.0030x over previous
import sys

import numpy as np

try:
    from concourse import bacc, bass, tile, mybir
    from concourse.bass_utils import run_bass_kernel_spmd
except ImportError:  # pragma: no cover
    sys.path.insert(0, "/opt/trn_rl_repo")
    from concourse import bacc, bass, tile, mybir
    from concourse.bass_utils import run_bass_kernel_spmd

B = 256        # batch
N = 16384      # neurons
DEG = 32       # fan-in per neuron
NCORES = 8
NPC = N // NCORES      # neurons per core = 2048
CH = 64                # neurons per chunk
NCHUNK = NPC // CH     # 32
IPC = CH * DEG         # gather indices per chunk = 2048
SW = IPC // 16         # wrapped idx cols per chunk = 128
G = 2                  # batch groups of 128

F32 = mybir.dt.float32
I16 = mybir.dt.int16

_CACHE = {}
TRACE = False
LAST_EXEC_TIME_NS = None
LAST_RESULTS = None


def _build():
    nc = bacc.Bacc("TRN2", target_bir_lowering=False)
    x_il = nc.dram_tensor([128, G * N], F32, kind="ExternalInput")
    idx_w = nc.dram_tensor([128, NCHUNK * SW], I16, kind="ExternalInput")
    w_wr = nc.dram_tensor([NCHUNK, IPC], F32, kind="ExternalInput")
    sel_d = nc.dram_tensor([NCHUNK, NCHUNK * 128], F32, kind="ExternalInput")
    out_d = nc.dram_tensor([128, NCHUNK * G * CH], F32, kind="ExternalOutput")

    with tile.TileContext(nc) as tc:
        with (
            tc.tile_pool(name="xp", bufs=1) as xp,
            tc.tile_pool(name="cst", bufs=1) as cst,
            tc.tile_pool(name="idxp", bufs=4) as idxp,
            tc.tile_pool(name="gath", bufs=2) as gath,
            tc.tile_pool(name="red", bufs=2) as red,
            tc.tile_pool(name="outp", bufs=2) as outp,
            tc.tile_pool(name="psum", bufs=2, space=bass.MemorySpace.PSUM) as psump,
        ):
            xs = xp.tile([128, G * N], F32)
            ws = cst.tile([NCHUNK, IPC], F32)
            sel = cst.tile([NCHUNK, NCHUNK * 128], F32)

            # bulk x load over all three queues; small constants on HWDGE
            qs = (nc.sync, nc.scalar, nc.gpsimd)
            nseg = 3
            seg = G * N // nseg
            bounds = [0, seg, 2 * seg, G * N]
            for q in range(nseg):
                qs[q].dma_start(xs[:, bounds[q]:bounds[q + 1]],
                                x_il[:, bounds[q]:bounds[q + 1]])
            nc.sync.dma_start(ws[:], w_wr[:])
            nc.scalar.dma_start(sel[:], sel_d[:])

            xs3 = xs[:].rearrange("p (n g) -> p n g", g=G)
            for c in range(NCHUNK):
                it = idxp.tile([128, SW], I16)
                (nc.sync if c % 2 else nc.scalar).dma_start(
                    it[:], idx_w[:, c * SW:(c + 1) * SW])
                # broadcast w row c to 128 partitions: pw = onehot_c.T @ ws
                pw = psump.tile([128, IPC], F32)
                for k in range(IPC // 512):
                    nc.tensor.matmul(
                        pw[:, k * 512:(k + 1) * 512],
                        sel[0:NCHUNK, c * 128:(c + 1) * 128],
                        ws[0:NCHUNK, k * 512:(k + 1) * 512],
                        start=True, stop=True,
                    )
                # gather: gt[p, j, g] = x[g*128+p, L[c*IPC+j]]
                gt = gath.tile([128, IPC * G], F32)
                nc.gpsimd.ap_gather(
                    gt[:], xs3, it[:],
                    channels=128, num_elems=N, d=G, num_idxs=IPC,
                )
                # gt *= w (in place), w broadcast over g
                gt3 = gt[:].rearrange("p (j g) -> p j g", g=G)
                pwb = pw[:].unsqueeze(2).broadcast_to([128, IPC, G])
                nc.vector.tensor_tensor(gt3, gt3, pwb, mybir.AluOpType.mult)
                # reduce over fan-in d=32 (stride-2 inner): [128, i=64, g=2]
                ro = red.tile([128, CH * G], F32)
                nc.vector.tensor_reduce(
                    ro[:].rearrange("p (i g) -> p i g", g=G),
                    gt[:].rearrange("p (i d g) -> p i g d", d=DEG, g=G),
                    axis=mybir.AxisListType.X,
                    op=mybir.AluOpType.add,
                )
                # relu + deinterleave to g-major [p, g*64+i]
                oo = outp.tile([128, CH * G], F32)
                nc.scalar.activation(
                    oo[:].rearrange("p (g i) -> p i g", i=CH),
                    ro[:].rearrange("p (i g) -> p i g", g=G),
                    func=mybir.ActivationFunctionType.Relu,
                )
                (nc.scalar if c % 2 else nc.sync).dma_start(
                    out_d[:, c * G * CH:(c + 1) * G * CH], oo[:])

    nc.finalize()
    names = dict(x_il=x_il.name, idx_w=idx_w.name, w_wr=w_wr.name,
                 sel=sel_d.name, out=out_d.name)
    return nc, names


def _prep(x, w, idx, names):
    x = np.ascontiguousarray(np.asarray(x, dtype=np.float32))
    w = np.asarray(w, dtype=np.float32)
    idx = np.asarray(idx)
    # x_il[p, n*2+g] = x[g*128+p, n]
    x_il = np.ascontiguousarray(
        x.reshape(G, 128, N).transpose(1, 2, 0).reshape(128, G * N))
    sel = np.zeros((NCHUNK, NCHUNK * 128), dtype=np.float32)
    for c in range(NCHUNK):
        sel[c, c * 128:(c + 1) * 128] = 1.0
    in_maps = []
    for m in range(NCORES):
        sl = slice(m * NPC, (m + 1) * NPC)
        li = np.ascontiguousarray(idx[sl]).astype(np.int16).reshape(-1)
        iw = np.ascontiguousarray(np.tile(li.reshape(NCHUNK * SW, 16).T, (8, 1)))
        w_wr = np.ascontiguousarray(w[sl].reshape(NCHUNK, IPC))
        in_maps.append({names["x_il"]: x_il, names["idx_w"]: iw,
                        names["w_wr"]: w_wr, names["sel"]: sel})
    return in_maps


def kernel(x, w, idx):
    global LAST_EXEC_TIME_NS, LAST_RESULTS
    if "prog" not in _CACHE:
        _CACHE["prog"] = _build()
    nc, names = _CACHE["prog"]
    in_maps = _prep(x, w, idx, names)
    res = run_bass_kernel_spmd(nc, in_maps, core_ids=list(range(NCORES)),
                               trace=TRACE)
    LAST_EXEC_TIME_NS = res.exec_time_ns
    LAST_RESULTS = res
    outs = []
    for m in range(NCORES):
        o = np.asarray(res.results[m][names["out"]])
        o = o.reshape(128, NCHUNK, G, CH).transpose(2, 0, 1, 3).reshape(B, NPC)
        outs.append(o)
    return np.ascontiguousarray(np.concatenate(outs, axis=1)).astype(np.float32)


# revision 21
# speedup vs baseline: 1.0638x; 1.0605x over previous
import sys

import numpy as np

try:
    from concourse import bacc, bass, tile, mybir
    from concourse.bass_utils import run_bass_kernel_spmd
except ImportError:  # pragma: no cover
    sys.path.insert(0, "/opt/trn_rl_repo")
    from concourse import bacc, bass, tile, mybir
    from concourse.bass_utils import run_bass_kernel_spmd

B = 256        # batch
N = 16384      # neurons
DEG = 32       # fan-in per neuron
NCORES = 8
NPC = N // NCORES      # neurons per core = 2048
CH = 64                # neurons per chunk
NCHUNK = NPC // CH     # 32
IPC = CH * DEG         # gather indices per chunk = 2048
SW = IPC // 16         # wrapped idx cols per chunk = 128
G = 2                  # batch groups of 128

F32 = mybir.dt.float32
BF16 = mybir.dt.bfloat16
I16 = mybir.dt.int16

_CACHE = {}
TRACE = False
LAST_EXEC_TIME_NS = None
LAST_RESULTS = None


def _build():
    nc = bacc.Bacc("TRN2", target_bir_lowering=False)
    x_il = nc.dram_tensor([128, G * N], BF16, kind="ExternalInput")
    idx_w = nc.dram_tensor([128, NCHUNK * SW], I16, kind="ExternalInput")
    w_wr = nc.dram_tensor([NCHUNK, IPC], F32, kind="ExternalInput")
    sel_d = nc.dram_tensor([NCHUNK, NCHUNK * 128], F32, kind="ExternalInput")
    out_d = nc.dram_tensor([128, NCHUNK * G * CH], F32, kind="ExternalOutput")

    with tile.TileContext(nc) as tc:
        with (
            tc.tile_pool(name="xp", bufs=1) as xp,
            tc.tile_pool(name="cst", bufs=1) as cst,
            tc.tile_pool(name="idxp", bufs=4) as idxp,
            tc.tile_pool(name="gath", bufs=2) as gath,
            tc.tile_pool(name="mpool", bufs=2) as mpool,
            tc.tile_pool(name="red", bufs=2) as red,
            tc.tile_pool(name="outp", bufs=2) as outp,
            tc.tile_pool(name="psum", bufs=2, space=bass.MemorySpace.PSUM) as psump,
        ):
            xs = xp.tile([128, G * N], BF16)
            ws = cst.tile([NCHUNK, IPC], F32)
            sel = cst.tile([NCHUNK, NCHUNK * 128], F32)

            # bulk x load over all three queues; small constants on HWDGE
            qs = (nc.sync, nc.scalar, nc.gpsimd)
            nseg = 3
            seg = G * N // nseg
            bounds = [0, seg, 2 * seg, G * N]
            for q in range(nseg):
                qs[q].dma_start(xs[:, bounds[q]:bounds[q + 1]],
                                x_il[:, bounds[q]:bounds[q + 1]])
            nc.sync.dma_start(ws[:], w_wr[:])
            nc.scalar.dma_start(sel[:], sel_d[:])

            xs3 = xs[:].rearrange("p (n g) -> p n g", g=G)
            for c in range(NCHUNK):
                it = idxp.tile([128, SW], I16)
                (nc.sync if c % 2 else nc.scalar).dma_start(
                    it[:], idx_w[:, c * SW:(c + 1) * SW])
                # broadcast w row c to 128 partitions: pw = onehot_c.T @ ws
                pw = psump.tile([128, IPC], F32)
                for k in range(IPC // 512):
                    nc.tensor.matmul(
                        pw[:, k * 512:(k + 1) * 512],
                        sel[0:NCHUNK, c * 128:(c + 1) * 128],
                        ws[0:NCHUNK, k * 512:(k + 1) * 512],
                        start=True, stop=True,
                    )
                # gather: gt[p, j, g] = x[g*128+p, L[c*IPC+j]]
                gt = gath.tile([128, IPC * G], BF16)
                nc.gpsimd.ap_gather(
                    gt[:], xs3, it[:],
                    channels=128, num_elems=N, d=G, num_idxs=IPC,
                )
                # mt = gt * w (f32 accumulate), w broadcast over g
                mt = mpool.tile([128, IPC * G], F32)
                gt3 = gt[:].rearrange("p (j g) -> p j g", g=G)
                mt3 = mt[:].rearrange("p (j g) -> p j g", g=G)
                pwb = pw[:].unsqueeze(2).broadcast_to([128, IPC, G])
                nc.vector.tensor_tensor(mt3, gt3, pwb, mybir.AluOpType.mult)
                # reduce over fan-in d=32 (stride-2 inner): [128, i=64, g=2]
                ro = red.tile([128, CH * G], F32)
                nc.vector.tensor_reduce(
                    ro[:].rearrange("p (i g) -> p i g", g=G),
                    mt[:].rearrange("p (i d g) -> p i g d", d=DEG, g=G),
                    axis=mybir.AxisListType.X,
                    op=mybir.AluOpType.add,
                )
                # relu + deinterleave to g-major [p, g*64+i]
                oo = outp.tile([128, CH * G], F32)
                nc.scalar.activation(
                    oo[:].rearrange("p (g i) -> p i g", i=CH),
                    ro[:].rearrange("p (i g) -> p i g", g=G),
                    func=mybir.ActivationFunctionType.Relu,
                )
                (nc.scalar if c % 2 else nc.sync).dma_start(
                    out_d[:, c * G * CH:(c + 1) * G * CH], oo[:])

    nc.finalize()
    names = dict(x_il=x_il.name, idx_w=idx_w.name, w_wr=w_wr.name,
                 sel=sel_d.name, out=out_d.name)
    return nc, names


def _prep(x, w, idx, names):
    x = np.ascontiguousarray(np.asarray(x, dtype=np.float32))
    w = np.asarray(w, dtype=np.float32)
    idx = np.asarray(idx)
    # x_il[p, n*2+g] = x[g*128+p, n]
    import ml_dtypes
    x_il = np.ascontiguousarray(
        x.reshape(G, 128, N).transpose(1, 2, 0).reshape(128, G * N)
    ).astype(ml_dtypes.bfloat16)
    sel = np.zeros((NCHUNK, NCHUNK * 128), dtype=np.float32)
    for c in range(NCHUNK):
        sel[c, c * 128:(c + 1) * 128] = 1.0
    in_maps = []
    for m in range(NCORES):
        sl = slice(m * NPC, (m + 1) * NPC)
        li = np.ascontiguousarray(idx[sl]).astype(np.int16).reshape(-1)
        iw = np.ascontiguousarray(np.tile(li.reshape(NCHUNK * SW, 16).T, (8, 1)))
        w_wr = np.ascontiguousarray(w[sl].reshape(NCHUNK, IPC))
        in_maps.append({names["x_il"]: x_il, names["idx_w"]: iw,
                        names["w_wr"]: w_wr, names["sel"]: sel})
    return in_maps


def kernel(x, w, idx):
    global LAST_EXEC_TIME_NS, LAST_RESULTS
    if "prog" not in _CACHE:
        _CACHE["prog"] = _build()
    nc, names = _CACHE["prog"]
    in_maps = _prep(x, w, idx, names)
    res = run_bass_kernel_spmd(nc, in_maps, core_ids=list(range(NCORES)),
                               trace=TRACE)
    LAST_EXEC_TIME_NS = res.exec_time_ns
    LAST_RESULTS = res
    outs = []
    for m in range(NCORES):
        o = np.asarray(res.results[m][names["out"]])
        o = o.reshape(128, NCHUNK, G, CH).transpose(2, 0, 1, 3).reshape(B, NPC)
        outs.append(o)
    return np.ascontiguousarray(np.concatenate(outs, axis=1)).astype(np.float32)
